# revision 1
# baseline (speedup 1.0000x reference)
"""Axial attention (no softmax) on 8 TRN2 NeuronCores.

Problem: x (8, 64, 64, 1024) fp32; two self-attentions (16 heads, no
softmax, scale d**-0.5) along the H axis (w_qkv0/w_out0) and the W axis
(w_qkv1/w_out1); output is their sum.

Sharding: data-parallel over batch B=8 -> one batch slab per core,
weights replicated. Each core computes both axial passes for its slab;
no collectives.

Per-core kernel structure (all matmuls bf16, fp32 PSUM accumulate):
  tokens t = h*64 + w (h-major), NT = 4096 per slab.
  For each pass (H-axis then W-axis), in chunks of 8 sequences
  (CH = 512 tokens, chunk token order is sequence-major):
    1. DMA natural x tiles [128 tok, 1024], PE-transpose to
       xT [128 d, 512 tok] tiles (8 k-tiles per chunk).
    2. qkT[m] = (Wqk[:, m-block]).T @ xT  -> [128 qk-dim, 512 tok]
       (16 m-tiles, 8 k accumulation steps each; q scaled by 1/32).
    3. v[tb] = x @ Wv -> [128 tok, 1024] natural layout (4 tok-blocks).
    4. Per (head-pair j, seq-pair sp): 4-way 64x64 tile_position packs:
       A^T = kT.T @ qT   (4 matmuls into one PSUM tile)
       O^T = v.T  @ A^T  (4 matmuls into one PSUM tile)
       assembling OT[j] [128 d, 512 tok].
    5. y = OT.T @ Wout -> [128 tok, 512] fp32; pass H writes out
       directly, pass W gpsimd-DMA-accumulates (out = oh + ow).
"""

import numpy as np
import ml_dtypes
from contextlib import ExitStack

from concourse.bass_utils import run_bass_kernel_spmd
from concourse import bacc, mybir, tile
from concourse.masks import make_identity

BF16 = mybir.dt.bfloat16
F32 = mybir.dt.float32

B = 8
D = 1024
NT = 4096          # tokens per core (64*64)
CH = 512           # chunk tokens (8 sequences of 64)
NCHUNK = NT // CH  # 8
KB = D // 128      # 8 contraction blocks
SCALE = 1.0 / 32.0  # 1024 ** -0.5

_BUILD_CACHE = {}
STAGE_MAP = {}


class _TensorProxy:
    """Records which pipeline stage emitted each PE instruction (for
    trace attribution in the perf harness)."""

    def __init__(self, te):
        self._te = te
        self.stage = "?"

    def matmul(self, *a, **kw):
        r = self._te.matmul(*a, **kw)
        STAGE_MAP[r.ins.name] = self.stage
        return r

    def transpose(self, *a, **kw):
        r = self._te.transpose(*a, **kw)
        STAGE_MAP[r.ins.name] = self.stage
        return r


def build(n_chunks=NCHUNK, passes=(0, 1)):
    key = (n_chunks, tuple(passes))
    if key in _BUILD_CACHE:
        return _BUILD_CACHE[key]

    nc = bacc.Bacc("TRN2", target_bir_lowering=False, debug=False)
    x = nc.dram_tensor("x", [NT, D], BF16, kind="ExternalInput")
    wqk = [nc.dram_tensor(f"wqk{p}", [D, 2 * D], BF16, kind="ExternalInput")
           for p in range(2)]
    wv = [nc.dram_tensor(f"wv{p}", [D, D], BF16, kind="ExternalInput")
          for p in range(2)]
    wo = [nc.dram_tensor(f"wo{p}", [D, D], BF16, kind="ExternalInput")
          for p in range(2)]
    out = nc.dram_tensor("out", [NT, D], F32, kind="ExternalOutput")

    xg = x.rearrange("(h w) d -> w h d", w=64)    # pass-H gather view
    og = out.rearrange("(h w) d -> w h d", w=64)  # pass-H scatter view

    with tile.TileContext(nc) as tc, ExitStack() as ctx:
        def pool(name, bufs, space="SBUF"):
            return ctx.enter_context(
                tc.tile_pool(name=name, bufs=bufs, space=space))

        p_id = pool("ident", 1)
        p_wqk = pool("wqk", 16)
        p_wv = pool("wv", 8)
        p_wo = pool("wo", 8)
        p_xn = pool("xn", 8)
        p_xt = pool("xt", 16)
        p_qkt = pool("qkt", 20)
        p_v = pool("v", 8)
        p_sa = pool("sa", 10)
        p_ot = pool("ot", 16)
        p_y = pool("y", 4)
        # PSUM budget: 8 banks total (each tile is padded to one bank).
        # Row-tiled 64x64 matmul packs need the two row tiles' outputs in
        # DIFFERENT banks (concurrent row tiles may not share a bank).
        ps_big = pool("psb", 3, "PSUM")    # [128, 512] f32 qkv/y groups
        ps_att = pool("psatt", 5, "PSUM")  # transpose + A^T/O^T halves

        te = _TensorProxy(nc.tensor)
        ident = p_id.tile([128, 128], BF16, name="ident")
        make_identity(nc, ident)

        # PE warm-up: ~5us of dummy matmuls while the first DMAs land,
        # so the HAM clock gate reaches 8/8 before real work starts.
        te.stage = "warm"
        warm_ps = ps_big.tile([128, 128], F32, tag="big", name="warm_ps")
        for _ in range(40):
            te.matmul(warm_ps[:], lhsT=ident[:], rhs=ident[:],
                      start=True, stop=True)

        for p in passes:
            if p == passes[0]:
                # prefetch chunk-0 x tiles ahead of the weight stream
                pre_xns = []
                engs = (nc.sync, nc.scalar, nc.gpsimd)
                for tb in range(4):
                    xn = p_xn.tile([128, D], BF16, tag="xn", name=f"xn_pre_{p}_{tb}")
                    if p == 1:
                        engs[tb % 3].dma_start(xn[:], x[tb * 128:(tb + 1) * 128, :])
                    else:
                        # one DMA per w-row, spread across engines
                        engs[(2 * tb) % 3].dma_start(
                            xn[0:64, :], xg[tb * 2, :, :])
                        engs[(2 * tb + 1) % 3].dma_start(
                            xn[64:128, :], xg[tb * 2 + 1, :, :])
                    pre_xns.append(xn)
            else:
                pre_xns = None
            wqk_t = []
            for k in range(KB):
                t = p_wqk.tile([128, 2 * D], BF16, tag="wqk", name=f"wqk_{p}_{k}")
                nc.sync.dma_start(t[:], wqk[p][k * 128:(k + 1) * 128, :])
                wqk_t.append(t)
            wv_t = []
            for k in range(KB):
                t = p_wv.tile([128, D], BF16, tag="wv", name=f"wv_{p}_{k}")
                nc.scalar.dma_start(t[:], wv[p][k * 128:(k + 1) * 128, :])
                wv_t.append(t)
            wo_t = []
            for k in range(KB):
                t = p_wo.tile([128, D], BF16, tag="wo", name=f"wo_{p}_{k}")
                nc.scalar.dma_start(t[:], wo[p][k * 128:(k + 1) * 128, :])
                wo_t.append(t)

            for c in range(n_chunks):
                # 1. load natural x tiles, PE-transpose into xT k-tiles.
                # All 4 transposes of one k-block go into one [128, 512]
                # PSUM tile (one bank, one copy out).
                xt = [p_xt.tile([128, CH], BF16, tag="xt", name=f"xt_{p}_{c}_{i}") for i in range(KB)]
                if c == 0 and pre_xns is not None:
                    xns = pre_xns
                else:
                    xns = []
                    engs = (nc.sync, nc.scalar, nc.gpsimd)
                    for tb in range(4):
                        xn = p_xn.tile([128, D], BF16, tag="xn", name=f"xn_{p}_{c}_{tb}")
                        if p == 1:
                            t0 = c * CH + tb * 128
                            engs[(c * 4 + tb) % 3].dma_start(
                                xn[:], x[t0:t0 + 128, :])
                        else:
                            w0 = c * 8 + tb * 2
                            engs[(c * 8 + 2 * tb) % 3].dma_start(
                                xn[0:64, :], xg[w0, :, :])
                            engs[(c * 8 + 2 * tb + 1) % 3].dma_start(
                                xn[64:128, :], xg[w0 + 1, :, :])
                        xns.append(xn)
                te.stage = "transp"
                for k in range(KB):
                    pt = ps_big.tile([128, CH], BF16, tag="big", name=f"pt_{p}_{c}_{k}")
                    for tb in range(4):
                        te.transpose(
                            pt[:, tb * 128:(tb + 1) * 128],
                            xns[tb][:, k * 128:(k + 1) * 128], ident[:])
                    nc.vector.tensor_copy(xt[k][:], pt[:])

                # 2. qkT projection: 16 m-tiles, accumulate over 8 k-blocks
                qkt = [p_qkt.tile([128, CH], BF16, tag="qkt", name=f"qkt_{p}_{c}_{i}")
                       for i in range(16)]
                te.stage = "qkT"
                for m in range(16):
                    pq = ps_big.tile([128, CH], F32, tag="big", name=f"pq_{p}_{c}_{m}")
                    for k in range(KB):
                        te.matmul(
                            pq[:],
                            lhsT=wqk_t[k][:, m * 128:(m + 1) * 128],
                            rhs=xt[k][:],
                            start=(k == 0), stop=(k == KB - 1))
                    nc.vector.tensor_copy(qkt[m][:], pq[:])

                # 3. v projection, natural [tok, d] layout
                v_t = [p_v.tile([128, D], BF16, tag="v", name=f"v_{p}_{c}_{i}") for i in range(4)]
                te.stage = "v"
                for tb in range(4):
                    for n2 in range(2):
                        pv = ps_big.tile([128, CH], F32, tag="big", name=f"pv_{p}_{c}_{tb}_{n2}")
                        for k in range(KB):
                            te.matmul(
                                pv[:],
                                lhsT=xt[k][:, tb * 128:(tb + 1) * 128],
                                rhs=wv_t[k][:, n2 * 512:(n2 + 1) * 512],
                                start=(k == 0), stop=(k == KB - 1))
                        nc.vector.tensor_copy(
                            v_t[tb][:, n2 * 512:(n2 + 1) * 512], pv[:])

                # 4. attention, batched per head-pair j: all 8 sequences'
                # A^T (and O^T) land in one PSUM bank per PE row-tile
                # (row tiles must not share a bank), 16 dense 64x64
                # matmuls per bank pair, then one copy per bank.
                # paE = head 2j (row tile 0), paO = head 2j+1 (row tile 1);
                # layout: rows (s%2)*64, cols (s//2)*64.
                te.stage = "att"
                # Software pipeline: emit A(j+1), A(j+2) between A(j) and
                # O(j) so the PSUM->SBUF copies of A(j) are fully off the
                # PE critical path.
                ot = [p_ot.tile([128, CH], BF16, tag="ot", name=f"ot_{p}_{c}_{i}") for i in range(8)]

                def emit_A(j):
                    te.stage = "attA"
                    kq = qkt[8 + j]
                    qq = qkt[j]
                    paE = ps_att.tile([128, 256], F32, tag="att", name=f"paE_{p}_{c}_{j}")
                    paO = ps_att.tile([128, 256], F32, tag="att", name=f"paO_{p}_{c}_{j}")
                    for s in range(8):
                        rp = (s % 2) * 64
                        fc = (s // 2) * 64
                        ssl = slice(s * 64, (s + 1) * 64)
                        te.matmul(
                            paE[rp:rp + 64, fc:fc + 64],
                            lhsT=kq[0:64, ssl], rhs=qq[0:64, ssl],
                            start=True, stop=True, tile_position=(0, rp))
                        te.matmul(
                            paO[rp:rp + 64, fc:fc + 64],
                            lhsT=kq[64:128, ssl], rhs=qq[64:128, ssl],
                            start=True, stop=True, tile_position=(64, rp))
                    saE = p_sa.tile([128, 256], BF16, tag="sa", name=f"saE_{p}_{c}_{j}")
                    saO = p_sa.tile([128, 256], BF16, tag="sa", name=f"saO_{p}_{c}_{j}")
                    nc.scalar.copy(saE[:], paE[:])
                    nc.vector.tensor_copy(saO[:], paO[:])
                    return saE, saO

                def emit_O(j, saE, saO):
                    te.stage = "attO"
                    poS0 = ps_att.tile([128, 256], F32, tag="att", name=f"poS0_{p}_{c}_{j}")
                    poS1 = ps_att.tile([128, 256], F32, tag="att", name=f"poS1_{p}_{c}_{j}")
                    h0 = slice((2 * j) * 64, (2 * j + 1) * 64)
                    h1 = slice((2 * j + 1) * 64, (2 * j + 2) * 64)
                    for s in range(8):
                        rv = (s % 2) * 64
                        fc = (s // 2) * 64
                        vv = v_t[s // 2]
                        dst = poS0 if s % 2 == 0 else poS1
                        te.matmul(
                            dst[0:64, fc:fc + 64],
                            lhsT=vv[rv:rv + 64, h0],
                            rhs=saE[rv:rv + 64, fc:fc + 64],
                            start=True, stop=True, tile_position=(rv, 0))
                        te.matmul(
                            dst[64:128, fc:fc + 64],
                            lhsT=vv[rv:rv + 64, h1],
                            rhs=saO[rv:rv + 64, fc:fc + 64],
                            start=True, stop=True, tile_position=(rv, 64))
                    otv = ot[j].rearrange("p (s2 par t) -> p par s2 t", par=2, t=64)
                    po0v = poS0.rearrange("p (s2 t) -> p s2 t", t=64)
                    po1v = poS1.rearrange("p (s2 t) -> p s2 t", t=64)
                    nc.vector.tensor_copy(otv[:, 0], po0v)
                    nc.vector.tensor_copy(otv[:, 1], po1v)

                pend = []
                for j in range(8):
                    sa_pair = emit_A(j)
                    if len(pend) >= 2:
                        oj = pend.pop(0)
                        emit_O(oj[0], oj[1], oj[2])
                    pend.append((j, sa_pair[0], sa_pair[1]))
                for oj in pend:
                    emit_O(oj[0], oj[1], oj[2])

                te.stage = "y"
                for tb in range(4):
                    ysb = p_y.tile([128, D], F32, tag="y", name=f"y_{p}_{c}_{tb}")
                    for n2 in range(2):
                        py = ps_big.tile([128, CH], F32, tag="big", name=f"py_{p}_{c}_{tb}_{n2}")
                        for i in range(KB):
                            k = (i + tb * 2 + n2) % KB
                            te.matmul(
                                py[:],
                                lhsT=ot[k][:, tb * 128:(tb + 1) * 128],
                                rhs=wo_t[k][:, n2 * 512:(n2 + 1) * 512],
                                start=(i == 0), stop=(i == KB - 1))
                        nc.vector.tensor_copy(
                            ysb[:, n2 * 512:(n2 + 1) * 512], py[:])
                    if p == 1:
                        t0 = c * CH + tb * 128
                        nc.gpsimd.dma_start(
                            out[t0:t0 + 128, :], ysb[:],
                            accum_op=mybir.AluOpType.add)
                    else:
                        w0 = c * 8 + tb * 2
                        yeng = nc.sync if tb % 2 == 0 else nc.scalar
                        yeng.dma_start(og[w0:w0 + 2, :, :], ysb[:])
    nc.compile()
    _BUILD_CACHE[key] = nc
    return nc


def _prep_inputs(x, w_qkv0, w_out0, w_qkv1, w_out1):
    bf = ml_dtypes.bfloat16
    xb = np.ascontiguousarray(x.reshape(B, NT, D)).astype(bf)
    common = {}
    for p, (wqkv, wout) in enumerate(((w_qkv0, w_out0), (w_qkv1, w_out1))):
        wqk_s = np.ascontiguousarray(wqkv[:, :2 * D]).copy()
        wqk_s[:, :D] *= SCALE  # fold q scale into weights (2^-5, exact)
        common[f"wqk{p}"] = wqk_s.astype(bf)
        common[f"wv{p}"] = np.ascontiguousarray(wqkv[:, 2 * D:]).astype(bf)
        common[f"wo{p}"] = np.ascontiguousarray(wout).astype(bf)
    return [{"x": xb[b], **common} for b in range(B)]


def kernel(x, w_qkv0, w_out0, w_qkv1, w_out1, trace=False, tmpdir=None):
    nc = build()
    in_maps = _prep_inputs(x, w_qkv0, w_out0, w_qkv1, w_out1)
    res = run_bass_kernel_spmd(nc, in_maps, core_ids=list(range(B)),
                               trace=trace, tmpdir=tmpdir)
    outs = np.stack([res.results[b]["out"] for b in range(B)])
    outs = outs.reshape(B, 64, 64, D)
    kernel.last_result = res
    return outs



# revision 3
# speedup vs baseline: 1.0509x; 1.0509x over previous
"""Axial attention (no softmax) on 8 TRN2 NeuronCores.

Problem: x (8, 64, 64, 1024) fp32; two self-attentions (16 heads, no
softmax, scale d**-0.5) along the H axis (w_qkv0/w_out0) and the W axis
(w_qkv1/w_out1); output is their sum.

Sharding: data-parallel over batch B=8 -> one batch slab per core,
weights replicated. Each core computes both axial passes for its slab;
no collectives.

v2 design (vs v1 baseline at 1155us):
  - x is transposed on the HOST into xT0 (w-major tokens, pass H) and
    xT1 (h-major tokens, pass W), both [1024 d, 4096 tok] bf16. This
    deletes all 512 PE transposes + their PSUM/DVE traffic.
  - 16 global chunks (8 per pass) of 512 tokens. Steady-state emission
    interleaves chunk c's attention (tiny 64x64 quadrant-packed MMs)
    with chunk c+1's qkT projection (dense FD=512 MMs): the PE array
    never has a low-activity window, so the HAM clock gate stays at
    8/8 (v1 re-throttled to half clock every chunk) and attention's
    PSUM-copy stalls are filled with dense work.
  - Weight pools are rings sized to prefetch pass-1 weights during
    pass 0 (wqk split into half-tiles so the ring grain is finer).
  Per-chunk PE work: qkT 16 m-groups x 8 k (FD=512), v 8 groups x 8 k,
  att 8 head-pairs x 32 64x64 MMs (4-way tile_position packs), y 8
  groups x 8 k. Pass H scatters output rows, pass W accumulates via
  gpsimd DMA (out = oh + ow).
"""

import numpy as np
import ml_dtypes
from contextlib import ExitStack

from concourse.bass_utils import run_bass_kernel_spmd
from concourse import bacc, mybir, tile
from concourse.masks import make_identity

BF16 = mybir.dt.bfloat16
F32 = mybir.dt.float32

B = 8
D = 1024
NT = 4096          # tokens per core (64*64)
CH = 512           # chunk tokens (8 sequences of 64)
NCHUNK = NT // CH  # 8 per pass
NC_ALL = 2 * NCHUNK
KB = D // 128      # 8 contraction blocks
SCALE = 1.0 / 32.0  # 1024 ** -0.5

_BUILD_CACHE = {}
STAGE_MAP = {}


class _TensorProxy:
    """Records which pipeline stage emitted each PE instruction (for
    trace attribution in the perf harness)."""

    def __init__(self, te):
        self._te = te
        self.stage = "?"

    def matmul(self, *a, **kw):
        r = self._te.matmul(*a, **kw)
        STAGE_MAP[r.ins.name] = self.stage
        return r


def build():
    key = "v2"
    if key in _BUILD_CACHE:
        return _BUILD_CACHE[key]

    nc = bacc.Bacc("TRN2", target_bir_lowering=False, debug=False)
    xT = [nc.dram_tensor(f"xT{p}", [D, NT], BF16, kind="ExternalInput")
          for p in range(2)]
    wqk = [nc.dram_tensor(f"wqk{p}", [D, 2 * D], BF16, kind="ExternalInput")
           for p in range(2)]
    wv = [nc.dram_tensor(f"wv{p}", [D, D], BF16, kind="ExternalInput")
          for p in range(2)]
    wo = [nc.dram_tensor(f"wo{p}", [D, D], BF16, kind="ExternalInput")
          for p in range(2)]
    out = nc.dram_tensor("out", [NT, D], F32, kind="ExternalOutput")
    og = out.rearrange("(h w) d -> w h d", w=64)  # pass-H scatter view

    with tile.TileContext(nc) as tc, ExitStack() as ctx:
        def pool(name, bufs, space="SBUF"):
            return ctx.enter_context(
                tc.tile_pool(name=name, bufs=bufs, space=space))

        p_id = pool("ident", 1)
        p_wqk = pool("wqk", 24)   # [128,1024] halves; 16 live + 8 prefetch
        p_wv = pool("wv", 12)
        p_wo = pool("wo", 12)
        p_xt = pool("xt", 24)     # 3 chunks in flight
        p_qkt = pool("qkt", 32)   # 2 chunks
        p_v = pool("v", 8)        # 2 chunks
        p_sa = pool("sa", 10)
        p_ot = pool("ot", 16)     # 2 chunks
        p_y = pool("y", 4)
        # PSUM: 8 banks. big = [128,512] f32 (1 bank each), att =
        # [128,256] f32 (padded to a bank). 2 + 6 = 8.
        ps_big = pool("psb", 2, "PSUM")
        ps_att = pool("psatt", 6, "PSUM")

        te = _TensorProxy(nc.tensor)
        ident = p_id.tile([128, 128], BF16, name="ident")
        make_identity(nc, ident)

        DMA_ENGS = (nc.sync, nc.scalar, nc.gpsimd)

        # ---- weight tiles (all passes up front; ring pools gate the
        # actual DMA start so pass-1 tiles prefetch as pass 0 retires).
        wqk_t = {}   # (p, k, half) -> [128, 1024]
        wv_t = {}
        wo_t = {}
        for p in range(2):
            for k in range(KB):
                for h in range(2):
                    t = p_wqk.tile([128, D], BF16, tag="wqk",
                                   name=f"wqk_{p}_{k}_{h}")
                    DMA_ENGS[(2 * k + h) % 2].dma_start(
                        t[:], wqk[p][k * 128:(k + 1) * 128,
                                     h * D:(h + 1) * D])
                    wqk_t[(p, k, h)] = t
        for p in range(2):
            for k in range(KB):
                t = p_wv.tile([128, D], BF16, tag="wv", name=f"wv_{p}_{k}")
                nc.scalar.dma_start(t[:], wv[p][k * 128:(k + 1) * 128, :])
                wv_t[(p, k)] = t
                t = p_wo.tile([128, D], BF16, tag="wo", name=f"wo_{p}_{k}")
                nc.gpsimd.dma_start(t[:], wo[p][k * 128:(k + 1) * 128, :])
                wo_t[(p, k)] = t

        # ---- per-chunk state
        xt_t = {}    # c -> [k] tiles [128, 512]
        qkt_t = {}   # c -> [m] tiles (16), [128, 512] bf16
        v_t = {}     # c -> [tb] tiles (4), [128, 1024] bf16
        ot_t = {}    # c -> [j] tiles (8), [128, 512] bf16

        def emit_xt(c):
            p, lc = divmod(c, NCHUNK)
            ts = []
            for k in range(KB):
                t = p_xt.tile([128, CH], BF16, tag="xt", name=f"xt_{c}_{k}")
                DMA_ENGS[(c * KB + k) % 3].dma_start(
                    t[:], xT[p][k * 128:(k + 1) * 128,
                                lc * CH:(lc + 1) * CH])
                ts.append(t)
            xt_t[c] = ts

        def emit_qkT_group(c, m):
            p = c // NCHUNK
            te.stage = "qkT"
            if c not in qkt_t:
                qkt_t[c] = [None] * 16
            q = p_qkt.tile([128, CH], BF16, tag="qkt", name=f"qkt_{c}_{m}")
            pq = ps_big.tile([128, CH], F32, tag="big", name=f"pq_{c}_{m}")
            half, mm = divmod(m, 8)
            for i in range(KB):
                k = (i + m) % KB
                te.matmul(
                    pq[:],
                    lhsT=wqk_t[(p, k, half)][:, mm * 128:(mm + 1) * 128],
                    rhs=xt_t[c][k][:],
                    start=(i == 0), stop=(i == KB - 1))
            nc.vector.tensor_copy(q[:], pq[:])
            qkt_t[c][m] = q

        def emit_v_group(c, g):
            p = c // NCHUNK
            te.stage = "v"
            tb, n2 = divmod(g, 2)
            if c not in v_t:
                v_t[c] = [None] * 4
            if v_t[c][tb] is None:
                v_t[c][tb] = p_v.tile([128, D], BF16, tag="v",
                                      name=f"v_{c}_{tb}")
            pv = ps_big.tile([128, CH], F32, tag="big", name=f"pv_{c}_{g}")
            for i in range(KB):
                k = (i + g) % KB
                te.matmul(
                    pv[:],
                    lhsT=xt_t[c][k][:, tb * 128:(tb + 1) * 128],
                    rhs=wv_t[(p, k)][:, n2 * 512:(n2 + 1) * 512],
                    start=(i == 0), stop=(i == KB - 1))
            nc.vector.tensor_copy(v_t[c][tb][:, n2 * 512:(n2 + 1) * 512],
                                  pv[:])

        sa_t = {}    # (c, j) -> (saE, saO)

        def emit_A(c, j):
            te.stage = "attA"
            kq = qkt_t[c][8 + j]
            qq = qkt_t[c][j]
            paE = ps_att.tile([128, 256], F32, tag="att", name=f"paE_{c}_{j}")
            paO = ps_att.tile([128, 256], F32, tag="att", name=f"paO_{c}_{j}")
            for s in range(8):
                rp = (s % 2) * 64
                fc = (s // 2) * 64
                ssl = slice(s * 64, (s + 1) * 64)
                te.matmul(
                    paE[rp:rp + 64, fc:fc + 64],
                    lhsT=kq[0:64, ssl], rhs=qq[0:64, ssl],
                    start=True, stop=True, tile_position=(0, rp))
                te.matmul(
                    paO[rp:rp + 64, fc:fc + 64],
                    lhsT=kq[64:128, ssl], rhs=qq[64:128, ssl],
                    start=True, stop=True, tile_position=(64, rp))
            saE = p_sa.tile([128, 256], BF16, tag="sa", name=f"saE_{c}_{j}")
            saO = p_sa.tile([128, 256], BF16, tag="sa", name=f"saO_{c}_{j}")
            nc.scalar.copy(saE[:], paE[:])
            nc.vector.tensor_copy(saO[:], paO[:])
            sa_t[(c, j)] = (saE, saO)

        def emit_O(c, j):
            te.stage = "attO"
            saE, saO = sa_t.pop((c, j))
            if c not in ot_t:
                ot_t[c] = [None] * 8
            ot = p_ot.tile([128, CH], BF16, tag="ot", name=f"ot_{c}_{j}")
            ot_t[c][j] = ot
            poS0 = ps_att.tile([128, 256], F32, tag="att", name=f"poS0_{c}_{j}")
            poS1 = ps_att.tile([128, 256], F32, tag="att", name=f"poS1_{c}_{j}")
            h0 = slice((2 * j) * 64, (2 * j + 1) * 64)
            h1 = slice((2 * j + 1) * 64, (2 * j + 2) * 64)
            for s in range(8):
                rv = (s % 2) * 64
                fc = (s // 2) * 64
                vv = v_t[c][s // 2]
                dst = poS0 if s % 2 == 0 else poS1
                te.matmul(
                    dst[0:64, fc:fc + 64],
                    lhsT=vv[rv:rv + 64, h0],
                    rhs=saE[rv:rv + 64, fc:fc + 64],
                    start=True, stop=True, tile_position=(rv, 0))
                te.matmul(
                    dst[64:128, fc:fc + 64],
                    lhsT=vv[rv:rv + 64, h1],
                    rhs=saO[rv:rv + 64, fc:fc + 64],
                    start=True, stop=True, tile_position=(rv, 64))
            otv = ot.rearrange("p (s2 par t) -> p par s2 t", par=2, t=64)
            po0v = poS0.rearrange("p (s2 t) -> p s2 t", t=64)
            po1v = poS1.rearrange("p (s2 t) -> p s2 t", t=64)
            nc.vector.tensor_copy(otv[:, 0], po0v)
            nc.vector.tensor_copy(otv[:, 1], po1v)

        def emit_y_group(c, tb):
            p, lc = divmod(c, NCHUNK)
            te.stage = "y"
            ysb = p_y.tile([128, D], F32, tag="y", name=f"y_{c}_{tb}")
            for n2 in range(2):
                py = ps_big.tile([128, CH], F32, tag="big",
                                 name=f"py_{c}_{tb}_{n2}")
                for i in range(KB):
                    k = (i + tb * 2 + n2) % KB
                    te.matmul(
                        py[:],
                        lhsT=ot_t[c][k][:, tb * 128:(tb + 1) * 128],
                        rhs=wo_t[(p, k)][:, n2 * 512:(n2 + 1) * 512],
                        start=(i == 0), stop=(i == KB - 1))
                nc.vector.tensor_copy(
                    ysb[:, n2 * 512:(n2 + 1) * 512], py[:])
            if p == 1:
                t0 = lc * CH + tb * 128
                nc.gpsimd.dma_start(out[t0:t0 + 128, :], ysb[:],
                                    accum_op=mybir.AluOpType.add)
            else:
                w0 = lc * 8 + tb * 2
                yeng = nc.sync if tb % 2 == 0 else nc.scalar
                yeng.dma_start(og[w0:w0 + 2, :, :], ysb[:])

        # ---- prologue
        emit_xt(0)
        emit_xt(1)

        # PE warm-up: dummy matmuls while the first DMAs land, so the
        # HAM clock gate reaches 8/8 before real work starts.
        te.stage = "warm"
        warm_ps = ps_big.tile([128, CH], F32, tag="big", name="warm_ps")
        for _ in range(40):
            te.matmul(warm_ps[:, 0:128], lhsT=ident[:], rhs=ident[:],
                      start=True, stop=True)

        for m in range(16):
            emit_qkT_group(0, m)
        for g in range(8):
            emit_v_group(0, g)

        # ---- steady state
        for c in range(NC_ALL):
            if c + 2 < NC_ALL:
                emit_xt(c + 2)
            for j in range(8):
                emit_A(c, j)
                if c + 1 < NC_ALL:
                    emit_qkT_group(c + 1, 2 * j)
                    emit_qkT_group(c + 1, 2 * j + 1)
                if j >= 1:
                    emit_O(c, j - 1)
            emit_O(c, 7)
            for tb in range(4):
                emit_y_group(c, tb)
            if c + 1 < NC_ALL:
                for g in range(8):
                    emit_v_group(c + 1, g)
    nc.compile()
    _BUILD_CACHE[key] = nc
    return nc


def _prep_inputs(x, w_qkv0, w_out0, w_qkv1, w_out1):
    bf = ml_dtypes.bfloat16
    xb = np.ascontiguousarray(x.reshape(B, 64, 64, D))
    # pass H (axis 0): tokens w-major (t = w*64 + h); pass W: h-major.
    xT0 = np.ascontiguousarray(xb.transpose(0, 3, 2, 1)).reshape(B, D, NT)
    xT1 = np.ascontiguousarray(xb.transpose(0, 3, 1, 2)).reshape(B, D, NT)
    xT0 = xT0.astype(bf)
    xT1 = xT1.astype(bf)
    common = {}
    for p, (wqkv, wout) in enumerate(((w_qkv0, w_out0), (w_qkv1, w_out1))):
        wqk_s = np.ascontiguousarray(wqkv[:, :2 * D]).copy()
        wqk_s[:, :D] *= SCALE  # fold q scale into weights (2^-5, exact)
        common[f"wqk{p}"] = wqk_s.astype(bf)
        common[f"wv{p}"] = np.ascontiguousarray(wqkv[:, 2 * D:]).astype(bf)
        common[f"wo{p}"] = np.ascontiguousarray(wout).astype(bf)
    return [{"xT0": xT0[b], "xT1": xT1[b], **common} for b in range(B)]


def kernel(x, w_qkv0, w_out0, w_qkv1, w_out1, trace=False, tmpdir=None):
    nc = build()
    in_maps = _prep_inputs(x, w_qkv0, w_out0, w_qkv1, w_out1)
    res = run_bass_kernel_spmd(nc, in_maps, core_ids=list(range(B)),
                               trace=trace, tmpdir=tmpdir)
    outs = np.stack([res.results[b]["out"] for b in range(B)])
    outs = outs.reshape(B, 64, 64, D)
    kernel.last_result = res
    return outs


# revision 4
# speedup vs baseline: 1.1070x; 1.0534x over previous
"""Axial attention (no softmax) on 8 TRN2 NeuronCores.

Problem: x (8, 64, 64, 1024) fp32; two self-attentions (16 heads, no
softmax, scale d**-0.5) along the H axis (w_qkv0/w_out0) and the W axis
(w_qkv1/w_out1); output is their sum.

Sharding: data-parallel over batch B=8 -> one batch slab per core,
weights replicated. Each core computes both axial passes for its slab;
no collectives.

v3 design (v1 baseline 1155us, v2 1099us):
  - x is transposed on the HOST into xT0 (w-major tokens, pass H) and
    xT1 (h-major tokens, pass W), both [1024 d, 4096 tok] bf16 (no PE
    transposes).
  - 16 global chunks (8 per pass) of 512 tokens; chunk c's attention
    (tiny 64x64 quadrant-packed MMs) is interleaved with chunk c+1's
    qkT projection (dense FD=512 MMs) so the PE array never has a
    low-activity window (keeps the HAM clock gate at 8/8).
  - sync/scalar are pure DMA engines (HWDGE); ALL PSUM evacuation is
    on vector. v2 lesson: a dma_start whose ring-slot semaphore
    resolves far in the future blocks the strict-FIFO engine, stalling
    everything behind it -- so pass-1 weight loads are emitted exactly
    when their ring slots are about to free (c=6: wqk1 h0, c=7:
    wqk1 h1 + wv1, c=8: wo1), with ring sizes aligned to emission
    order.
  - Startup: h0-half weights first + 48 FD=512 warm-up matmuls bridge
    the initial DMA window at full clock.
"""

import numpy as np
import ml_dtypes
from contextlib import ExitStack

from concourse.bass_utils import run_bass_kernel_spmd
from concourse import bacc, mybir, tile
from concourse.masks import make_identity

BF16 = mybir.dt.bfloat16
F32 = mybir.dt.float32

B = 8
D = 1024
NT = 4096          # tokens per core (64*64)
CH = 512           # chunk tokens (8 sequences of 64)
NCHUNK = NT // CH  # 8 per pass
NC_ALL = 2 * NCHUNK
KB = D // 128      # 8 contraction blocks
SCALE = 1.0 / 32.0  # 1024 ** -0.5

_BUILD_CACHE = {}
STAGE_MAP = {}


class _TensorProxy:
    """Records which pipeline stage emitted each PE instruction (for
    trace attribution in the perf harness)."""

    def __init__(self, te):
        self._te = te
        self.stage = "?"

    def matmul(self, *a, **kw):
        r = self._te.matmul(*a, **kw)
        STAGE_MAP[r.ins.name] = self.stage
        return r


def build():
    key = "v3"
    if key in _BUILD_CACHE:
        return _BUILD_CACHE[key]

    nc = bacc.Bacc("TRN2", target_bir_lowering=False, debug=False)
    xT = [nc.dram_tensor(f"xT{p}", [D, NT], BF16, kind="ExternalInput")
          for p in range(2)]
    wqk = [nc.dram_tensor(f"wqk{p}", [D, 2 * D], BF16, kind="ExternalInput")
           for p in range(2)]
    wv = [nc.dram_tensor(f"wv{p}", [D, D], BF16, kind="ExternalInput")
          for p in range(2)]
    wo = [nc.dram_tensor(f"wo{p}", [D, D], BF16, kind="ExternalInput")
          for p in range(2)]
    out = nc.dram_tensor("out", [NT, D], F32, kind="ExternalOutput")
    og = out.rearrange("(h w) d -> w h d", w=64)  # pass-H scatter view

    with tile.TileContext(nc) as tc, ExitStack() as ctx:
        def pool(name, bufs, space="SBUF"):
            return ctx.enter_context(
                tc.tile_pool(name=name, bufs=bufs, space=space))

        p_id = pool("ident", 2)
        p_wqk = pool("wqk", 16)   # [128,1024] halves; ring = one pass
        p_wv = pool("wv", 8)
        p_wo = pool("wo", 8)
        p_xt = pool("xt", 24)     # 3 chunks in flight
        p_qkt = pool("qkt", 32)   # 2 chunks
        p_v = pool("v", 8)        # 2 chunks
        p_sa = pool("sa", 10)
        p_ot = pool("ot", 16)     # 2 chunks
        p_y = pool("y", 8)
        # PSUM: 8 banks. big = [128,512] f32 (1 bank each), att =
        # [128,256] f32 (padded to a bank). 2 + 6 = 8.
        ps_big = pool("psb", 2, "PSUM")
        ps_att = pool("psatt", 6, "PSUM")

        te = _TensorProxy(nc.tensor)
        ident = p_id.tile([128, 128], BF16, name="ident")
        make_identity(nc, ident)
        wrm = p_id.tile([128, CH], BF16, name="wrm")
        for i in range(4):
            nc.vector.tensor_copy(wrm[:, i * 128:(i + 1) * 128], ident[:])

        # ---- weight tile emission helpers (h-major so the q|k halves
        # needed first arrive first; rings sized so a pass-1 push waits
        # at most a few us when emitted at the right iteration).
        wqk_t = {}   # (p, k, half) -> [128, 1024]
        wv_t = {}
        wo_t = {}

        def emit_wqk(p, half):
            for k in range(KB):
                t = p_wqk.tile([128, D], BF16, tag="wqk",
                               name=f"wqk_{p}_{k}_{half}")
                (nc.sync if k % 2 == 0 else nc.scalar).dma_start(
                    t[:], wqk[p][k * 128:(k + 1) * 128,
                                 half * D:(half + 1) * D])
                wqk_t[(p, k, half)] = t

        def emit_wv(p):
            for k in range(KB):
                t = p_wv.tile([128, D], BF16, tag="wv", name=f"wv_{p}_{k}")
                nc.scalar.dma_start(t[:], wv[p][k * 128:(k + 1) * 128, :])
                wv_t[(p, k)] = t

        def emit_wo(p):
            for k in range(KB):
                t = p_wo.tile([128, D], BF16, tag="wo", name=f"wo_{p}_{k}")
                nc.sync.dma_start(t[:], wo[p][k * 128:(k + 1) * 128, :])
                wo_t[(p, k)] = t

        # ---- per-chunk state
        DMA_ENGS = (nc.sync, nc.scalar, nc.gpsimd)
        xt_t = {}    # c -> [k] tiles [128, 512]
        qkt_t = {}   # c -> [m] tiles (16), [128, 512] bf16
        v_t = {}     # c -> [tb] tiles (4), [128, 1024] bf16
        ot_t = {}    # c -> [j] tiles (8), [128, 512] bf16

        def emit_xt(c):
            p, lc = divmod(c, NCHUNK)
            ts = []
            for k in range(KB):
                t = p_xt.tile([128, CH], BF16, tag="xt", name=f"xt_{c}_{k}")
                DMA_ENGS[(c * KB + k) % 3].dma_start(
                    t[:], xT[p][k * 128:(k + 1) * 128,
                                lc * CH:(lc + 1) * CH])
                ts.append(t)
            xt_t[c] = ts

        def emit_qkT_group(c, m):
            p = c // NCHUNK
            te.stage = "qkT"
            if c not in qkt_t:
                qkt_t[c] = [None] * 16
            q = p_qkt.tile([128, CH], BF16, tag="qkt", name=f"qkt_{c}_{m}")
            pq = ps_big.tile([128, CH], F32, tag="big", name=f"pq_{c}_{m}")
            half, mm = divmod(m, 8)
            for i in range(KB):
                k = (i + m) % KB
                te.matmul(
                    pq[:],
                    lhsT=wqk_t[(p, k, half)][:, mm * 128:(mm + 1) * 128],
                    rhs=xt_t[c][k][:],
                    start=(i == 0), stop=(i == KB - 1))
            nc.vector.tensor_copy(q[:], pq[:])
            qkt_t[c][m] = q

        def emit_v_group(c, g):
            p = c // NCHUNK
            te.stage = "v"
            tb, n2 = divmod(g, 2)
            if c not in v_t:
                v_t[c] = [None] * 4
            if v_t[c][tb] is None:
                v_t[c][tb] = p_v.tile([128, D], BF16, tag="v",
                                      name=f"v_{c}_{tb}")
            pv = ps_big.tile([128, CH], F32, tag="big", name=f"pv_{c}_{g}")
            for i in range(KB):
                k = (i + g) % KB
                te.matmul(
                    pv[:],
                    lhsT=xt_t[c][k][:, tb * 128:(tb + 1) * 128],
                    rhs=wv_t[(p, k)][:, n2 * 512:(n2 + 1) * 512],
                    start=(i == 0), stop=(i == KB - 1))
            nc.vector.tensor_copy(v_t[c][tb][:, n2 * 512:(n2 + 1) * 512],
                                  pv[:])

        sa_t = {}    # (c, j) -> (saE, saO)

        def emit_A(c, j):
            te.stage = "attA"
            kq = qkt_t[c][8 + j]
            qq = qkt_t[c][j]
            paE = ps_att.tile([128, 256], F32, tag="att", name=f"paE_{c}_{j}")
            paO = ps_att.tile([128, 256], F32, tag="att", name=f"paO_{c}_{j}")
            for s in range(8):
                rp = (s % 2) * 64
                fc = (s // 2) * 64
                ssl = slice(s * 64, (s + 1) * 64)
                te.matmul(
                    paE[rp:rp + 64, fc:fc + 64],
                    lhsT=kq[0:64, ssl], rhs=qq[0:64, ssl],
                    start=True, stop=True, tile_position=(0, rp))
                te.matmul(
                    paO[rp:rp + 64, fc:fc + 64],
                    lhsT=kq[64:128, ssl], rhs=qq[64:128, ssl],
                    start=True, stop=True, tile_position=(64, rp))
            saE = p_sa.tile([128, 256], BF16, tag="sa", name=f"saE_{c}_{j}")
            saO = p_sa.tile([128, 256], BF16, tag="sa", name=f"saO_{c}_{j}")
            nc.vector.tensor_copy(saE[:], paE[:])
            nc.vector.tensor_copy(saO[:], paO[:])
            sa_t[(c, j)] = (saE, saO)

        def emit_O(c, j):
            te.stage = "attO"
            saE, saO = sa_t.pop((c, j))
            if c not in ot_t:
                ot_t[c] = [None] * 8
            ot = p_ot.tile([128, CH], BF16, tag="ot", name=f"ot_{c}_{j}")
            ot_t[c][j] = ot
            poS0 = ps_att.tile([128, 256], F32, tag="att", name=f"poS0_{c}_{j}")
            poS1 = ps_att.tile([128, 256], F32, tag="att", name=f"poS1_{c}_{j}")
            h0 = slice((2 * j) * 64, (2 * j + 1) * 64)
            h1 = slice((2 * j + 1) * 64, (2 * j + 2) * 64)
            for s in range(8):
                rv = (s % 2) * 64
                fc = (s // 2) * 64
                vv = v_t[c][s // 2]
                dst = poS0 if s % 2 == 0 else poS1
                te.matmul(
                    dst[0:64, fc:fc + 64],
                    lhsT=vv[rv:rv + 64, h0],
                    rhs=saE[rv:rv + 64, fc:fc + 64],
                    start=True, stop=True, tile_position=(rv, 0))
                te.matmul(
                    dst[64:128, fc:fc + 64],
                    lhsT=vv[rv:rv + 64, h1],
                    rhs=saO[rv:rv + 64, fc:fc + 64],
                    start=True, stop=True, tile_position=(rv, 64))
            otv = ot.rearrange("p (s2 par t) -> p par s2 t", par=2, t=64)
            po0v = poS0.rearrange("p (s2 t) -> p s2 t", t=64)
            po1v = poS1.rearrange("p (s2 t) -> p s2 t", t=64)
            nc.vector.tensor_copy(otv[:, 0], po0v)
            nc.vector.tensor_copy(otv[:, 1], po1v)

        def emit_y_group(c, tb):
            p, lc = divmod(c, NCHUNK)
            te.stage = "y"
            ysb = p_y.tile([128, D], F32, tag="y", name=f"y_{c}_{tb}")
            for n2 in range(2):
                py = ps_big.tile([128, CH], F32, tag="big",
                                 name=f"py_{c}_{tb}_{n2}")
                for i in range(KB):
                    k = (i + tb * 2 + n2) % KB
                    te.matmul(
                        py[:],
                        lhsT=ot_t[c][k][:, tb * 128:(tb + 1) * 128],
                        rhs=wo_t[(p, k)][:, n2 * 512:(n2 + 1) * 512],
                        start=(i == 0), stop=(i == KB - 1))
                nc.vector.tensor_copy(
                    ysb[:, n2 * 512:(n2 + 1) * 512], py[:])
            if p == 1:
                t0 = lc * CH + tb * 128
                nc.gpsimd.dma_start(out[t0:t0 + 128, :], ysb[:],
                                    accum_op=mybir.AluOpType.add)
            else:
                w0 = lc * 8 + tb * 2
                yeng = nc.sync if tb % 2 == 0 else nc.scalar
                yeng.dma_start(og[w0:w0 + 2, :, :], ysb[:])

        # ---- prologue
        emit_xt(0)
        emit_xt(1)
        emit_wqk(0, 0)
        emit_wqk(0, 1)
        emit_wv(0)
        emit_wo(0)

        # PE warm-up: FD=512 dummy matmuls bridge the initial weight/x
        # DMA window so the HAM clock gate reaches 8/8 before real work.
        te.stage = "warm"
        warm_ps = ps_big.tile([128, CH], F32, tag="big", name="warm_ps")
        for _ in range(48):
            te.matmul(warm_ps[:], lhsT=ident[:], rhs=wrm[:],
                      start=True, stop=True)

        for m in range(16):
            emit_qkT_group(0, m)
        for g in range(8):
            emit_v_group(0, g)

        # ---- steady state
        for c in range(NC_ALL):
            if c + 2 < NC_ALL:
                emit_xt(c + 2)
            if c == 6:
                emit_wqk(1, 0)
            elif c == 7:
                emit_wqk(1, 1)
                emit_wv(1)
            elif c == 8:
                emit_wo(1)
            for j in range(8):
                emit_A(c, j)
                if j >= 1:
                    emit_O(c, j - 1)
                if c + 1 < NC_ALL:
                    emit_qkT_group(c + 1, 2 * j)
                    emit_qkT_group(c + 1, 2 * j + 1)
            emit_O(c, 7)
            for tb in range(4):
                emit_y_group(c, tb)
            if c + 1 < NC_ALL:
                for g in range(8):
                    emit_v_group(c + 1, g)
    nc.compile()
    _BUILD_CACHE[key] = nc
    return nc


def _prep_inputs(x, w_qkv0, w_out0, w_qkv1, w_out1):
    bf = ml_dtypes.bfloat16
    xb = np.ascontiguousarray(x.reshape(B, 64, 64, D))
    # pass H (axis 0): tokens w-major (t = w*64 + h); pass W: h-major.
    xT0 = np.ascontiguousarray(xb.transpose(0, 3, 2, 1)).reshape(B, D, NT)
    xT1 = np.ascontiguousarray(xb.transpose(0, 3, 1, 2)).reshape(B, D, NT)
    xT0 = xT0.astype(bf)
    xT1 = xT1.astype(bf)
    common = {}
    for p, (wqkv, wout) in enumerate(((w_qkv0, w_out0), (w_qkv1, w_out1))):
        wqk_s = np.ascontiguousarray(wqkv[:, :2 * D]).copy()
        wqk_s[:, :D] *= SCALE  # fold q scale into weights (2^-5, exact)
        common[f"wqk{p}"] = wqk_s.astype(bf)
        common[f"wv{p}"] = np.ascontiguousarray(wqkv[:, 2 * D:]).astype(bf)
        common[f"wo{p}"] = np.ascontiguousarray(wout).astype(bf)
    return [{"xT0": xT0[b], "xT1": xT1[b], **common} for b in range(B)]


def kernel(x, w_qkv0, w_out0, w_qkv1, w_out1, trace=False, tmpdir=None):
    nc = build()
    in_maps = _prep_inputs(x, w_qkv0, w_out0, w_qkv1, w_out1)
    res = run_bass_kernel_spmd(nc, in_maps, core_ids=list(range(B)),
                               trace=trace, tmpdir=tmpdir)
    outs = np.stack([res.results[b]["out"] for b in range(B)])
    outs = outs.reshape(B, 64, 64, D)
    kernel.last_result = res
    return outs


# revision 6
# speedup vs baseline: 1.1446x; 1.0339x over previous
"""Axial attention (no softmax) on 8 TRN2 NeuronCores.

Problem: x (8, 64, 64, 1024) fp32; two self-attentions (16 heads, no
softmax, scale d**-0.5) along the H axis (w_qkv0/w_out0) and the W axis
(w_qkv1/w_out1); output is their sum.

Sharding: data-parallel over batch B=8 -> one batch slab per core,
weights replicated. Each core computes both axial passes for its slab;
no collectives.

v3 design (v1 baseline 1155us, v2 1099us):
  - x is transposed on the HOST into xT0 (w-major tokens, pass H) and
    xT1 (h-major tokens, pass W), both [1024 d, 4096 tok] bf16 (no PE
    transposes).
  - 16 global chunks (8 per pass) of 512 tokens; chunk c's attention
    (tiny 64x64 quadrant-packed MMs) is interleaved with chunk c+1's
    qkT projection (dense FD=512 MMs) so the PE array never has a
    low-activity window (keeps the HAM clock gate at 8/8).
  - sync/scalar are pure DMA engines (HWDGE); ALL PSUM evacuation is
    on vector. v2 lesson: a dma_start whose ring-slot semaphore
    resolves far in the future blocks the strict-FIFO engine, stalling
    everything behind it -- so pass-1 weight loads are emitted exactly
    when their ring slots are about to free (c=6: wqk1 h0, c=7:
    wqk1 h1 + wv1, c=8: wo1), with ring sizes aligned to emission
    order.
  - Startup: h0-half weights first + 48 FD=512 warm-up matmuls bridge
    the initial DMA window at full clock.
"""

import numpy as np
import ml_dtypes
from contextlib import ExitStack

from concourse.bass_utils import run_bass_kernel_spmd
from concourse import bacc, mybir, tile
from concourse.masks import make_identity

BF16 = mybir.dt.bfloat16
F32 = mybir.dt.float32

B = 8
D = 1024
NT = 4096          # tokens per core (64*64)
CH = 512           # chunk tokens (8 sequences of 64)
NCHUNK = NT // CH  # 8 per pass
NC_ALL = 2 * NCHUNK
KB = D // 128      # 8 contraction blocks
SCALE = 1.0 / 32.0  # 1024 ** -0.5

_BUILD_CACHE = {}
STAGE_MAP = {}


class _TensorProxy:
    """Records which pipeline stage emitted each PE instruction (for
    trace attribution in the perf harness)."""

    def __init__(self, te):
        self._te = te
        self.stage = "?"

    def matmul(self, *a, **kw):
        r = self._te.matmul(*a, **kw)
        STAGE_MAP[r.ins.name] = self.stage
        return r


def build():
    key = "v4"
    if key in _BUILD_CACHE:
        return _BUILD_CACHE[key]

    nc = bacc.Bacc("TRN2", target_bir_lowering=False, debug=False)
    xT = [nc.dram_tensor(f"xT{p}", [D, NT], BF16, kind="ExternalInput")
          for p in range(2)]
    wqk = [nc.dram_tensor(f"wqk{p}", [D, 2 * D], BF16, kind="ExternalInput")
           for p in range(2)]
    wv = [nc.dram_tensor(f"wv{p}", [D, D], BF16, kind="ExternalInput")
          for p in range(2)]
    wo = [nc.dram_tensor(f"wo{p}", [D, D], BF16, kind="ExternalInput")
          for p in range(2)]
    out = nc.dram_tensor("out", [NT, D], F32, kind="ExternalOutput")
    og = out.rearrange("(h w) d -> w h d", w=64)  # pass-H scatter view

    with tile.TileContext(nc) as tc, ExitStack() as ctx:
        def pool(name, bufs, space="SBUF"):
            return ctx.enter_context(
                tc.tile_pool(name=name, bufs=bufs, space=space))

        p_id = pool("ident", 2)
        p_wqk = pool("wqk", 16)   # [128,1024] halves; ring = one pass
        p_wv = pool("wv", 8)
        p_wo = pool("wo", 8)
        p_xt = pool("xt", 24)     # 3 chunks in flight
        p_qkt = pool("qkt", 32)   # 2 chunks
        p_v = pool("v", 8)        # 2 chunks
        p_sa = pool("sa", 10)
        p_ot = pool("ot", 16)     # 2 chunks
        p_y = pool("y", 8)
        # PSUM: 8 banks. big = [128,512] f32 (1 bank each), att =
        # [128,256] f32 (padded to a bank). 2 + 6 = 8.
        ps_big = pool("psb", 2, "PSUM")
        ps_att = pool("psatt", 6, "PSUM")

        te = _TensorProxy(nc.tensor)
        ident = p_id.tile([128, 128], BF16, name="ident")
        make_identity(nc, ident)
        wrm = p_id.tile([128, CH], BF16, name="wrm")
        for i in range(4):
            nc.vector.tensor_copy(wrm[:, i * 128:(i + 1) * 128], ident[:])

        # ---- weight tile emission helpers (h-major so the q|k halves
        # needed first arrive first; rings sized so a pass-1 push waits
        # at most a few us when emitted at the right iteration).
        wqk_t = {}   # (p, k, half) -> [128, 1024]
        wv_t = {}
        wo_t = {}

        def emit_wqk(p, half):
            for k in range(KB):
                t = p_wqk.tile([128, D], BF16, tag="wqk",
                               name=f"wqk_{p}_{k}_{half}")
                (nc.sync if k % 2 == 0 else nc.scalar).dma_start(
                    t[:], wqk[p][k * 128:(k + 1) * 128,
                                 half * D:(half + 1) * D])
                wqk_t[(p, k, half)] = t

        def emit_wv(p):
            for k in range(KB):
                t = p_wv.tile([128, D], BF16, tag="wv", name=f"wv_{p}_{k}")
                nc.scalar.dma_start(t[:], wv[p][k * 128:(k + 1) * 128, :])
                wv_t[(p, k)] = t

        def emit_wo(p):
            for k in range(KB):
                t = p_wo.tile([128, D], BF16, tag="wo", name=f"wo_{p}_{k}")
                nc.sync.dma_start(t[:], wo[p][k * 128:(k + 1) * 128, :])
                wo_t[(p, k)] = t

        # ---- per-chunk state
        # Queue discipline: sync/scalar carry ONLY input traffic (xt +
        # weights) whose ring gates resolve iterations early; gpsimd
        # carries ALL output traffic (og scatter + accum), whose pushes
        # legitimately wait tens of us on ysb copies -- on a dedicated
        # queue that wait blocks nothing else.
        DMA_ENGS = (nc.sync, nc.scalar)
        xt_t = {}    # c -> [k] tiles [128, 512]
        qkt_t = {}   # c -> [m] tiles (16), [128, 512] bf16
        v_t = {}     # c -> [tb] tiles (4), [128, 1024] bf16
        ot_t = {}    # c -> [j] tiles (8), [128, 512] bf16

        def emit_xt(c):
            p, lc = divmod(c, NCHUNK)
            ts = []
            for k in range(KB):
                t = p_xt.tile([128, CH], BF16, tag="xt", name=f"xt_{c}_{k}")
                DMA_ENGS[k % 2].dma_start(
                    t[:], xT[p][k * 128:(k + 1) * 128,
                                lc * CH:(lc + 1) * CH])
                ts.append(t)
            xt_t[c] = ts

        def emit_qkT_group(c, m):
            p = c // NCHUNK
            te.stage = "qkT"
            if c not in qkt_t:
                qkt_t[c] = [None] * 16
            q = p_qkt.tile([128, CH], BF16, tag="qkt", name=f"qkt_{c}_{m}")
            pq = ps_big.tile([128, CH], F32, tag="big", name=f"pq_{c}_{m}")
            half, mm = divmod(m, 8)
            for i in range(KB):
                k = (i + m) % KB
                te.matmul(
                    pq[:],
                    lhsT=wqk_t[(p, k, half)][:, mm * 128:(mm + 1) * 128],
                    rhs=xt_t[c][k][:],
                    start=(i == 0), stop=(i == KB - 1))
            nc.vector.tensor_copy(q[:], pq[:])
            qkt_t[c][m] = q

        def emit_v_group(c, g):
            p = c // NCHUNK
            te.stage = "v"
            tb, n2 = divmod(g, 2)
            if c not in v_t:
                v_t[c] = [None] * 4
            if v_t[c][tb] is None:
                v_t[c][tb] = p_v.tile([128, D], BF16, tag="v",
                                      name=f"v_{c}_{tb}")
            pv = ps_big.tile([128, CH], F32, tag="big", name=f"pv_{c}_{g}")
            for i in range(KB):
                k = (i + g) % KB
                te.matmul(
                    pv[:],
                    lhsT=xt_t[c][k][:, tb * 128:(tb + 1) * 128],
                    rhs=wv_t[(p, k)][:, n2 * 512:(n2 + 1) * 512],
                    start=(i == 0), stop=(i == KB - 1))
            nc.vector.tensor_copy(v_t[c][tb][:, n2 * 512:(n2 + 1) * 512],
                                  pv[:])

        sa_t = {}    # (c, j) -> (saE, saO)

        def emit_A(c, j):
            te.stage = "attA"
            kq = qkt_t[c][8 + j]
            qq = qkt_t[c][j]
            paE = ps_att.tile([128, 256], F32, tag="att", name=f"paE_{c}_{j}")
            paO = ps_att.tile([128, 256], F32, tag="att", name=f"paO_{c}_{j}")
            for s in range(8):
                rp = (s % 2) * 64
                fc = (s // 2) * 64
                ssl = slice(s * 64, (s + 1) * 64)
                te.matmul(
                    paE[rp:rp + 64, fc:fc + 64],
                    lhsT=kq[0:64, ssl], rhs=qq[0:64, ssl],
                    start=True, stop=True, tile_position=(0, rp))
                te.matmul(
                    paO[rp:rp + 64, fc:fc + 64],
                    lhsT=kq[64:128, ssl], rhs=qq[64:128, ssl],
                    start=True, stop=True, tile_position=(64, rp))
            saE = p_sa.tile([128, 256], BF16, tag="sa", name=f"saE_{c}_{j}")
            saO = p_sa.tile([128, 256], BF16, tag="sa", name=f"saO_{c}_{j}")
            nc.vector.tensor_copy(saE[:], paE[:])
            nc.vector.tensor_copy(saO[:], paO[:])
            sa_t[(c, j)] = (saE, saO)

        def emit_O(c, j):
            te.stage = "attO"
            saE, saO = sa_t.pop((c, j))
            if c not in ot_t:
                ot_t[c] = [None] * 8
            ot = p_ot.tile([128, CH], BF16, tag="ot", name=f"ot_{c}_{j}")
            ot_t[c][j] = ot
            poS0 = ps_att.tile([128, 256], F32, tag="att", name=f"poS0_{c}_{j}")
            poS1 = ps_att.tile([128, 256], F32, tag="att", name=f"poS1_{c}_{j}")
            h0 = slice((2 * j) * 64, (2 * j + 1) * 64)
            h1 = slice((2 * j + 1) * 64, (2 * j + 2) * 64)
            for s in range(8):
                rv = (s % 2) * 64
                fc = (s // 2) * 64
                vv = v_t[c][s // 2]
                dst = poS0 if s % 2 == 0 else poS1
                te.matmul(
                    dst[0:64, fc:fc + 64],
                    lhsT=vv[rv:rv + 64, h0],
                    rhs=saE[rv:rv + 64, fc:fc + 64],
                    start=True, stop=True, tile_position=(rv, 0))
                te.matmul(
                    dst[64:128, fc:fc + 64],
                    lhsT=vv[rv:rv + 64, h1],
                    rhs=saO[rv:rv + 64, fc:fc + 64],
                    start=True, stop=True, tile_position=(rv, 64))
            otv = ot.rearrange("p (s2 par t) -> p par s2 t", par=2, t=64)
            po0v = poS0.rearrange("p (s2 t) -> p s2 t", t=64)
            po1v = poS1.rearrange("p (s2 t) -> p s2 t", t=64)
            nc.vector.tensor_copy(otv[:, 0], po0v)
            nc.vector.tensor_copy(otv[:, 1], po1v)

        def emit_y_group(c, tb):
            p, lc = divmod(c, NCHUNK)
            te.stage = "y"
            ysb = p_y.tile([128, D], F32, tag="y", name=f"y_{c}_{tb}")
            for n2 in range(2):
                py = ps_big.tile([128, CH], F32, tag="big",
                                 name=f"py_{c}_{tb}_{n2}")
                for i in range(KB):
                    k = (i + tb * 2 + n2) % KB
                    te.matmul(
                        py[:],
                        lhsT=ot_t[c][k][:, tb * 128:(tb + 1) * 128],
                        rhs=wo_t[(p, k)][:, n2 * 512:(n2 + 1) * 512],
                        start=(i == 0), stop=(i == KB - 1))
                nc.vector.tensor_copy(
                    ysb[:, n2 * 512:(n2 + 1) * 512], py[:])
            if p == 1:
                t0 = lc * CH + tb * 128
                nc.gpsimd.dma_start(out[t0:t0 + 128, :], ysb[:],
                                    accum_op=mybir.AluOpType.add)
            else:
                w0 = lc * 8 + tb * 2
                nc.gpsimd.dma_start(og[w0:w0 + 2, :, :], ysb[:])

        # ---- prologue
        emit_xt(0)
        emit_xt(1)
        emit_wqk(0, 0)
        emit_wqk(0, 1)
        emit_wv(0)
        emit_wo(0)

        # PE warm-up: FD=512 dummy matmuls bridge the initial weight/x
        # DMA window so the HAM clock gate reaches 8/8 before real work.
        te.stage = "warm"
        warm_ps = ps_big.tile([128, CH], F32, tag="big", name="warm_ps")
        for _ in range(48):
            te.matmul(warm_ps[:], lhsT=ident[:], rhs=wrm[:],
                      start=True, stop=True)

        for m in range(16):
            emit_qkT_group(0, m)
        for g in range(8):
            emit_v_group(0, g)

        # ---- steady state
        for c in range(NC_ALL):
            if c + 2 < NC_ALL:
                emit_xt(c + 2)
            if c == 6:
                emit_wqk(1, 0)
            elif c == 7:
                emit_wqk(1, 1)
                emit_wv(1)
            elif c == 8:
                emit_wo(1)
            for j in range(8):
                emit_A(c, j)
                if j >= 1:
                    emit_O(c, j - 1)
                if c + 1 < NC_ALL:
                    emit_qkT_group(c + 1, 2 * j)
                    emit_qkT_group(c + 1, 2 * j + 1)
            emit_O(c, 7)
            for tb in range(4):
                emit_y_group(c, tb)
            if c + 1 < NC_ALL:
                for g in range(8):
                    emit_v_group(c + 1, g)
    nc.compile()
    _BUILD_CACHE[key] = nc
    return nc


def _prep_inputs(x, w_qkv0, w_out0, w_qkv1, w_out1):
    bf = ml_dtypes.bfloat16
    xb = np.ascontiguousarray(x.reshape(B, 64, 64, D))
    # pass H (axis 0): tokens w-major (t = w*64 + h); pass W: h-major.
    xT0 = np.ascontiguousarray(xb.transpose(0, 3, 2, 1)).reshape(B, D, NT)
    xT1 = np.ascontiguousarray(xb.transpose(0, 3, 1, 2)).reshape(B, D, NT)
    xT0 = xT0.astype(bf)
    xT1 = xT1.astype(bf)
    common = {}
    for p, (wqkv, wout) in enumerate(((w_qkv0, w_out0), (w_qkv1, w_out1))):
        wqk_s = np.ascontiguousarray(wqkv[:, :2 * D]).copy()
        wqk_s[:, :D] *= SCALE  # fold q scale into weights (2^-5, exact)
        common[f"wqk{p}"] = wqk_s.astype(bf)
        common[f"wv{p}"] = np.ascontiguousarray(wqkv[:, 2 * D:]).astype(bf)
        common[f"wo{p}"] = np.ascontiguousarray(wout).astype(bf)
    return [{"xT0": xT0[b], "xT1": xT1[b], **common} for b in range(B)]


def kernel(x, w_qkv0, w_out0, w_qkv1, w_out1, trace=False, tmpdir=None):
    nc = build()
    in_maps = _prep_inputs(x, w_qkv0, w_out0, w_qkv1, w_out1)
    res = run_bass_kernel_spmd(nc, in_maps, core_ids=list(range(B)),
                               trace=trace, tmpdir=tmpdir)
    outs = np.stack([res.results[b]["out"] for b in range(B)])
    outs = outs.reshape(B, 64, 64, D)
    kernel.last_result = res
    return outs
